# revision 14
# baseline (speedup 1.0000x reference)
"""KMeans min-distance loss kernel for Trainium2 (8 NeuronCores, SPMD).

Problem: features [262144, 128] f32, centers [256, 128] f32.
  d2[n,k] = ||f_n||^2 + ||c_k||^2 - 2 f_n.c_k ; out = mean_n sqrt(min_k d2)

Sharding: data-parallel over N (32768 rows per core), centers replicated.
Each core returns [128] partial sums of min-distances; host reduces.

Per-core pipeline (bf16 compute, f32 accumulate):
  - SWDGE cast-DMA 1MB groups: f32 dram -> bf16 sbuf [128p, 16, 128]
  - PE transpose (bf16) chunks -> PSUM, batches of TG per bank
  - ACT evacuates PSUM -> SBUF twice: fT (copy) and fT2 (Square)
  - PE: fp8 DoubleRow rank-2 matmul preloads centered ||c||^2 (hi+lo
    split) into PSUM, then bf16 cross matmuls accumulate -2 f.c
  - PE: 1-col matmuls fT2.T @ ones -> f2 into a small PSUM tile
  - DVE: tensor_reduce min over k -> m; tensor_tensor add f2 + m
  - tail: ACT sqrt(d2 + mean_c2) with accum -> [128] sums -> DMA out
"""

import sys

for p in ("/opt/trn_rl_repo", "/opt/trn_rl_repo/concourse"):
    if p not in sys.path:
        sys.path.insert(0, p)

import numpy as np

N_TOTAL = 262144
K = 256
D = 128
N_CORES = 8
N_PER_CORE = N_TOTAL // N_CORES  # 32768
P = 128
CHUNKS = N_PER_CORE // P         # 256 chunks of 128 rows
G = 16                           # chunks per DMA group (1 MB f32 read)
GROUPS = CHUNKS // G             # 16
TG = 4                           # chunks per transpose/psum/reduce batch

_compiled = None


def _build():
    import concourse.bass as bass
    import concourse.bacc as bacc
    import concourse.tile as tile
    from concourse import mybir

    f32 = mybir.dt.float32
    bf16 = mybir.dt.bfloat16
    fp8 = mybir.dt.float8e4
    Alu = mybir.AluOpType
    Act = mybir.ActivationFunctionType

    nc = bacc.Bacc(
        "TRN2", target_bir_lowering=False, debug=False, num_devices=N_CORES
    )

    feats = nc.dram_tensor("features", [N_PER_CORE, D], f32, kind="ExternalInput").ap()
    ctneg2 = nc.dram_tensor("ctneg2", [D, K], bf16, kind="ExternalInput").ap()
    c2q = nc.dram_tensor("c2q", [1, 2, 2 * K], fp8, kind="ExternalInput").ap()
    ones2 = nc.dram_tensor("ones2", [1, 2, P], fp8, kind="ExternalInput").ap()
    onesb = nc.dram_tensor("onesb", [D, 1], bf16, kind="ExternalInput").ap()
    ident = nc.dram_tensor("ident", [P, P], bf16, kind="ExternalInput").ap()
    c2mean = nc.dram_tensor("c2mean", [P, 1], f32, kind="ExternalInput").ap()
    out = nc.dram_tensor("out", [P, 1], f32, kind="ExternalOutput").ap()

    with tile.TileContext(nc) as tc:
        with (
            tc.tile_pool(name="consts", bufs=1) as consts,
            tc.tile_pool(name="featg", bufs=3) as featg_pool,
            tc.tile_pool(name="featT", bufs=6) as featT_pool,
            tc.tile_pool(name="coll", bufs=1) as coll,
            tc.tile_pool(name="ptrans", bufs=2, space="PSUM") as ptrans_pool,
            tc.tile_pool(name="pf2p", bufs=2, space="PSUM") as pf2_pool,
            tc.tile_pool(name="pcross", bufs=2, space="PSUM") as pcross_pool,
        ):
            ct_s = consts.tile([D, K], bf16)
            nc.sync.dma_start(ct_s[:], ctneg2)
            c2q_s = consts.tile([1, 2, 2 * K], fp8)
            nc.sync.dma_start(c2q_s[:], c2q)
            ones_s = consts.tile([1, 2, P], fp8)
            nc.sync.dma_start(ones_s[:], ones2)
            onesb_s = consts.tile([D, 1], bf16)
            nc.sync.dma_start(onesb_s[:], onesb)
            id_s = consts.tile([P, P], bf16)
            nc.sync.dma_start(id_s[:], ident)
            c2m_s = consts.tile([P, 1], f32)
            nc.sync.dma_start(c2m_s[:], c2mean)

            m_coll = coll.tile([P, CHUNKS], f32)
            d2_coll = coll.tile([P, CHUNKS], f32)

            # HAM warm-up: the PE clock gate promotes 1.2->2.4 GHz only
            # after a fully-busy 3.4us window, and the steady-state batch
            # has just enough sem-wait bubbles that a cold start can stay
            # cold forever (bimodal 128us/194us runs). Burn ~64 dummy
            # matmuls into a scratch PSUM bank while the first feature
            # group DMA is still in flight: guarantees promotion, costs
            # no wall-clock (PE would be idle waiting on DMA anyway).
            warm = pcross_pool.tile([P, TG, K], f32, tag="px")
            for _ in range(64):
                nc.tensor.matmul(
                    warm[:, 0, :], id_s[:], ct_s[:],
                    start=True, stop=True, skip_group_check=True,
                )

            # features viewed as [group, partition, chunk-in-group, d].
            # Partition p takes G consecutive rows (one 8KB contiguous
            # descriptor per partition); chunk->row mapping is permuted,
            # which the order-invariant sum tolerates.
            fview = feats.rearrange("(g p c) d -> g p c d", p=P, c=G)

            # Software pipeline: batch i's cross/f2/reduce are issued after
            # batch i+1's transposes+preload, so PE never waits on the ACT
            # evacuations (keeps PE continuously busy -> full 2.4 GHz).
            pend = None  # (fT, fT2, px, pf2, i)

            def flush(pend):
                fT, fT2, px, pf2, i = pend
                for j in range(TG):
                    nc.tensor.matmul(
                        px[:, j, :], fT[:, bass.ts(j, P)], ct_s[:],
                        start=False, stop=(j == TG - 1),
                        skip_group_check=True,
                    )
                for j in range(TG):
                    nc.tensor.matmul(
                        pf2[:, j : j + 1],
                        fT2[:, bass.ts(j, P)],
                        onesb_s[:],
                        start=True, stop=True,
                        skip_group_check=True,
                    )
                nc.vector.tensor_reduce(
                    out=m_coll[:, i : i + TG],
                    in_=px[:],
                    axis=mybir.AxisListType.X,
                    op=Alu.min,
                )
                nc.vector.tensor_tensor(
                    out=d2_coll[:, i : i + TG],
                    in0=pf2,
                    in1=m_coll[:, i : i + TG],
                    op=Alu.add,
                )

            for g in range(GROUPS):
                fg = featg_pool.tile([P, G, D], bf16)
                nc.gpsimd.dma_start(fg[:], fview[g])  # SWDGE cast f32->bf16

                for cb in range(G // TG):
                    pt_t = ptrans_pool.tile([D, TG * P], bf16, tag="pt")
                    pt = pt_t[:]
                    pf2_t = pf2_pool.tile([P, TG], f32, tag="pf2")
                    pf2 = pf2_t[:]
                    for j in range(TG):
                        c = cb * TG + j
                        nc.tensor.transpose(
                            pt[:, bass.ts(j, P)], fg[:, c, :], id_s[:]
                        )
                    px = pcross_pool.tile([P, TG, K], f32, tag="px")
                    pxf = px[:].rearrange("p c k -> p (c k)")
                    for h in range(2):
                        nc.tensor.matmul(
                            pxf[:, bass.ts(h, 2 * K)],
                            ones_s[:],
                            c2q_s[:],
                            start=True, stop=False,
                            perf_mode=mybir.MatmulPerfMode.DoubleRow,
                            skip_group_check=True,
                        )
                    fT = featT_pool.tile([D, TG * P], bf16, tag="fT")
                    nc.scalar.copy(fT[:], pt)
                    fT2 = featT_pool.tile([D, TG * P], bf16, tag="fT2")
                    nc.scalar.activation(fT2[:], pt, Act.Square)

                    if pend is not None:
                        flush(pend)
                    pend = (fT, fT2, px, pf2, g * G + cb * TG)

            flush(pend)

            # tail: sums[p] = sum_i sqrt(d2[p,i] + c2mean)
            dist = coll.tile([P, CHUNKS], f32)
            sums = coll.tile([P, 1], f32)
            nc.scalar.activation(
                dist[:], d2_coll[:], Act.Sqrt, bias=c2m_s[:], accum_out=sums[:]
            )
            nc.sync.dma_start(out, sums[:])

    nc.compile()
    return nc


def _get_compiled():
    global _compiled
    if _compiled is None:
        _compiled = _build()
    return _compiled


def _make_aux(centers: np.ndarray):
    import ml_dtypes

    BF16 = ml_dtypes.bfloat16
    FP8 = ml_dtypes.float8_e4m3fn

    cen_bf = centers.astype(BF16)
    ctneg2 = np.ascontiguousarray(
        (-2.0 * cen_bf.astype(np.float32).T)
    ).astype(BF16)                                             # [D, K]
    c_eff = ctneg2.astype(np.float64) / -2.0
    c2 = (c_eff ** 2).sum(axis=0)                              # [K]
    c2m = float(c2.mean())
    c2c = (c2 - c2m).astype(np.float32)
    hi = c2c.astype(FP8)
    lo = (c2c - hi.astype(np.float32)).astype(FP8)
    c2q = np.zeros((1, 2, 2 * K), dtype=FP8)
    c2q[0, 0, :] = np.tile(hi, 2)
    c2q[0, 1, :] = np.tile(lo, 2)
    ones2 = np.ones((1, 2, P), dtype=FP8)
    onesb = np.ones((D, 1), dtype=BF16)
    ident = np.eye(P, dtype=BF16)
    c2mean = np.full((P, 1), c2m, dtype=np.float32)
    return ctneg2, c2q, ones2, onesb, ident, c2mean


def _make_in_maps(features: np.ndarray, centers: np.ndarray):
    ctneg2, c2q, ones2, onesb, ident, c2mean = _make_aux(centers)
    return [
        {
            "features": features[c * N_PER_CORE : (c + 1) * N_PER_CORE],
            "ctneg2": ctneg2,
            "c2q": c2q,
            "ones2": ones2,
            "onesb": onesb,
            "ident": ident,
            "c2mean": c2mean,
        }
        for c in range(N_CORES)
    ]


def kernel(features: np.ndarray, centers: np.ndarray) -> np.ndarray:
    features = np.ascontiguousarray(np.asarray(features, dtype=np.float32))
    centers = np.ascontiguousarray(np.asarray(centers, dtype=np.float32))
    assert features.shape == (N_TOTAL, D) and centers.shape == (K, D)

    from concourse.bass_utils import run_bass_kernel_spmd

    nc = _get_compiled()
    in_maps = _make_in_maps(features, centers)
    res = run_bass_kernel_spmd(nc, in_maps, list(range(N_CORES)))
    total = 0.0
    for r in res.results:
        total += np.sum(r["out"].astype(np.float64))
    return np.float32(total / N_TOTAL)


if __name__ == "__main__":
    rng = np.random.default_rng(0)
    f = rng.standard_normal((N_TOTAL, D), dtype=np.float32)
    c = rng.standard_normal((K, D), dtype=np.float32)
    print(kernel(f, c))


# revision 15
# speedup vs baseline: 1.3122x; 1.3122x over previous
"""KMeans min-distance loss kernel for Trainium2 (8 NeuronCores, SPMD).

Problem: features [262144, 128] f32, centers [256, 128] f32.
  d2[n,k] = ||f_n||^2 + ||c_k||^2 - 2 f_n.c_k ; out = mean_n sqrt(min_k d2)

Sharding: data-parallel over N (32768 rows per core), centers replicated.
Each core returns [128] partial sums of min-distances; host reduces.

Per-core pipeline (bf16 compute, f32 accumulate):
  - SWDGE cast-DMA 1MB groups: f32 dram -> bf16 sbuf [128p, 16, 128]
  - PE transpose (bf16) chunks -> PSUM, batches of TG per bank
  - ACT evacuates PSUM -> SBUF twice: fT (copy) and fT2 (Square)
  - PE: fp8 DoubleRow rank-2 matmul preloads centered ||c||^2 (hi+lo
    split) into PSUM, then bf16 cross matmuls accumulate -2 f.c
  - PE: 1-col matmuls fT2.T @ ones -> f2 into a small PSUM tile
  - DVE: tensor_reduce min over k -> m; tensor_tensor add f2 + m
  - tail: ACT sqrt(d2 + mean_c2) with accum -> [128] sums -> DMA out
"""

import sys

for p in ("/opt/trn_rl_repo", "/opt/trn_rl_repo/concourse"):
    if p not in sys.path:
        sys.path.insert(0, p)

import numpy as np

N_TOTAL = 262144
K = 256
D = 128
N_CORES = 8
N_PER_CORE = N_TOTAL // N_CORES  # 32768
P = 128
CHUNKS = N_PER_CORE // P         # 256 chunks of 128 rows
G = 16                           # chunks per DMA group (1 MB f32 read)
GROUPS = CHUNKS // G             # 16
TG = 4                           # chunks per transpose/psum/reduce batch

_compiled = None


def _build():
    import concourse.bass as bass
    import concourse.bacc as bacc
    import concourse.tile as tile
    from concourse import mybir

    f32 = mybir.dt.float32
    bf16 = mybir.dt.bfloat16
    fp8 = mybir.dt.float8e4
    Alu = mybir.AluOpType
    Act = mybir.ActivationFunctionType

    nc = bacc.Bacc(
        "TRN2", target_bir_lowering=False, debug=False, num_devices=N_CORES
    )

    feats = nc.dram_tensor("features", [N_PER_CORE, D], f32, kind="ExternalInput").ap()
    ctneg2 = nc.dram_tensor("ctneg2", [D, K], bf16, kind="ExternalInput").ap()
    c2q = nc.dram_tensor("c2q", [1, 2, 2 * K], fp8, kind="ExternalInput").ap()
    ones2 = nc.dram_tensor("ones2", [1, 2, P], fp8, kind="ExternalInput").ap()
    onesb = nc.dram_tensor("onesb", [D, 1], bf16, kind="ExternalInput").ap()
    ident = nc.dram_tensor("ident", [P, P], bf16, kind="ExternalInput").ap()
    c2mean = nc.dram_tensor("c2mean", [P, 1], f32, kind="ExternalInput").ap()
    out = nc.dram_tensor("out", [P, 1], f32, kind="ExternalOutput").ap()

    with tile.TileContext(nc) as tc:
        with (
            tc.tile_pool(name="consts", bufs=1) as consts,
            tc.tile_pool(name="featg", bufs=3) as featg_pool,
            tc.tile_pool(name="featT", bufs=6) as featT_pool,
            tc.tile_pool(name="coll", bufs=1) as coll,
            tc.tile_pool(name="ptrans", bufs=2, space="PSUM") as ptrans_pool,
            tc.tile_pool(name="pf2p", bufs=2, space="PSUM") as pf2_pool,
            tc.tile_pool(name="pcross", bufs=2, space="PSUM") as pcross_pool,
        ):
            ct_s = consts.tile([D, K], bf16)
            nc.sync.dma_start(ct_s[:], ctneg2)
            c2q_s = consts.tile([1, 2, 2 * K], fp8)
            nc.sync.dma_start(c2q_s[:], c2q)
            ones_s = consts.tile([1, 2, P], fp8)
            nc.sync.dma_start(ones_s[:], ones2)
            onesb_s = consts.tile([D, 1], bf16)
            nc.sync.dma_start(onesb_s[:], onesb)
            id_s = consts.tile([P, P], bf16)
            nc.sync.dma_start(id_s[:], ident)
            c2m_s = consts.tile([P, 1], f32)
            nc.sync.dma_start(c2m_s[:], c2mean)

            m_coll = coll.tile([P, CHUNKS], f32)
            d2_coll = coll.tile([P, CHUNKS], f32)

            # HAM warm-up: the PE clock gate promotes 1.2->2.4 GHz only
            # after a fully-busy 3.4us window, and the steady-state batch
            # has just enough sem-wait bubbles that a cold start can stay
            # cold forever (bimodal 128us/194us runs). Burn ~64 dummy
            # matmuls into a scratch PSUM bank while the first feature
            # group DMA is still in flight: guarantees promotion, costs
            # no wall-clock (PE would be idle waiting on DMA anyway).
            warm = pcross_pool.tile([P, TG, K], f32, tag="px")
            for _ in range(64):
                nc.tensor.matmul(
                    warm[:, 0, :], id_s[:], ct_s[:],
                    start=True, stop=True, skip_group_check=True,
                )

            # features viewed as [group, partition, chunk-in-group, d].
            # Partition p takes G consecutive rows (one 8KB contiguous
            # descriptor per partition); chunk->row mapping is permuted,
            # which the order-invariant sum tolerates.
            fview = feats.rearrange("(g p c) d -> g p c d", p=P, c=G)

            # Software pipeline: batch i's cross/f2/reduce are issued after
            # batch i+1's transposes+preload, so PE never waits on the ACT
            # evacuations (keeps PE continuously busy -> full 2.4 GHz).
            pend = None  # (fT, fT2, px, pf2, i)

            def flush(pend):
                fT, fT2, px, pf2, i = pend
                for j in range(TG):
                    nc.tensor.matmul(
                        px[:, j, :], fT[:, bass.ts(j, P)], ct_s[:],
                        start=False, stop=(j == TG - 1),
                        skip_group_check=True,
                    )
                for j in range(TG):
                    nc.tensor.matmul(
                        pf2[:, j : j + 1],
                        fT2[:, bass.ts(j, P)],
                        onesb_s[:],
                        start=True, stop=True,
                        skip_group_check=True,
                    )
                nc.vector.tensor_reduce(
                    out=m_coll[:, i : i + TG],
                    in_=px[:],
                    axis=mybir.AxisListType.X,
                    op=Alu.min,
                )
                nc.vector.tensor_tensor(
                    out=d2_coll[:, i : i + TG],
                    in0=pf2,
                    in1=m_coll[:, i : i + TG],
                    op=Alu.add,
                )

            for g in range(GROUPS):
                fg = featg_pool.tile([P, G, D], bf16)
                nc.gpsimd.dma_start(fg[:], fview[g])  # SWDGE cast f32->bf16

                for cb in range(G // TG):
                    pt_t = ptrans_pool.tile([D, TG * P], bf16, tag="pt")
                    pt = pt_t[:]
                    pf2_t = pf2_pool.tile([P, TG], f32, tag="pf2")
                    pf2 = pf2_t[:]
                    for j in range(TG):
                        c = cb * TG + j
                        nc.tensor.transpose(
                            pt[:, bass.ts(j, P)], fg[:, c, :], id_s[:]
                        )
                    px = pcross_pool.tile([P, TG, K], f32, tag="px")
                    pxf = px[:].rearrange("p c k -> p (c k)")
                    for h in range(2):
                        nc.tensor.matmul(
                            pxf[:, bass.ts(h, 2 * K)],
                            ones_s[:],
                            c2q_s[:],
                            start=True, stop=False,
                            perf_mode=mybir.MatmulPerfMode.DoubleRow,
                            skip_group_check=True,
                        )
                    fT = featT_pool.tile([D, TG * P], bf16, tag="fT")
                    nc.scalar.copy(fT[:], pt)
                    fT2 = featT_pool.tile([D, TG * P], bf16, tag="fT2")
                    nc.scalar.activation(fT2[:], pt, Act.Square)

                    if pend is not None:
                        flush(pend)
                    pend = (fT, fT2, px, pf2, g * G + cb * TG)

                # Periodic re-warm: HAM only promotes 1.2->2.4 GHz on a
                # 100%-busy 3.4us window, which the steady state never
                # provides. If a utilization dip ever re-throttles the PE,
                # this burst re-promotes it within 4 groups instead of
                # leaving the whole rest of the run at half clock.
                if (g + 1) % 4 == 0 and g + 1 < GROUPS:
                    rw = ptrans_pool.tile([D, TG * P], bf16, tag="pt")
                    rwf = rw[:].bitcast(f32)
                    for _ in range(32):
                        nc.tensor.matmul(
                            rwf[:, 0:K], id_s[:], ct_s[:],
                            start=True, stop=True, skip_group_check=True,
                        )

            flush(pend)

            # tail: sums[p] = sum_i sqrt(d2[p,i] + c2mean)
            dist = coll.tile([P, CHUNKS], f32)
            sums = coll.tile([P, 1], f32)
            nc.scalar.activation(
                dist[:], d2_coll[:], Act.Sqrt, bias=c2m_s[:], accum_out=sums[:]
            )
            nc.sync.dma_start(out, sums[:])

    nc.compile()
    return nc


def _get_compiled():
    global _compiled
    if _compiled is None:
        _compiled = _build()
    return _compiled


def _make_aux(centers: np.ndarray):
    import ml_dtypes

    BF16 = ml_dtypes.bfloat16
    FP8 = ml_dtypes.float8_e4m3fn

    cen_bf = centers.astype(BF16)
    ctneg2 = np.ascontiguousarray(
        (-2.0 * cen_bf.astype(np.float32).T)
    ).astype(BF16)                                             # [D, K]
    c_eff = ctneg2.astype(np.float64) / -2.0
    c2 = (c_eff ** 2).sum(axis=0)                              # [K]
    c2m = float(c2.mean())
    c2c = (c2 - c2m).astype(np.float32)
    hi = c2c.astype(FP8)
    lo = (c2c - hi.astype(np.float32)).astype(FP8)
    c2q = np.zeros((1, 2, 2 * K), dtype=FP8)
    c2q[0, 0, :] = np.tile(hi, 2)
    c2q[0, 1, :] = np.tile(lo, 2)
    ones2 = np.ones((1, 2, P), dtype=FP8)
    onesb = np.ones((D, 1), dtype=BF16)
    ident = np.eye(P, dtype=BF16)
    c2mean = np.full((P, 1), c2m, dtype=np.float32)
    return ctneg2, c2q, ones2, onesb, ident, c2mean


def _make_in_maps(features: np.ndarray, centers: np.ndarray):
    ctneg2, c2q, ones2, onesb, ident, c2mean = _make_aux(centers)
    return [
        {
            "features": features[c * N_PER_CORE : (c + 1) * N_PER_CORE],
            "ctneg2": ctneg2,
            "c2q": c2q,
            "ones2": ones2,
            "onesb": onesb,
            "ident": ident,
            "c2mean": c2mean,
        }
        for c in range(N_CORES)
    ]


def kernel(features: np.ndarray, centers: np.ndarray) -> np.ndarray:
    features = np.ascontiguousarray(np.asarray(features, dtype=np.float32))
    centers = np.ascontiguousarray(np.asarray(centers, dtype=np.float32))
    assert features.shape == (N_TOTAL, D) and centers.shape == (K, D)

    from concourse.bass_utils import run_bass_kernel_spmd

    nc = _get_compiled()
    in_maps = _make_in_maps(features, centers)
    res = run_bass_kernel_spmd(nc, in_maps, list(range(N_CORES)))
    total = 0.0
    for r in res.results:
        total += np.sum(r["out"].astype(np.float64))
    return np.float32(total / N_TOTAL)


if __name__ == "__main__":
    rng = np.random.default_rng(0)
    f = rng.standard_normal((N_TOTAL, D), dtype=np.float32)
    c = rng.standard_normal((K, D), dtype=np.float32)
    print(kernel(f, c))
